# revision 33
# baseline (speedup 1.0000x reference)
"""Bahdanau attention kernel for Trainium2, 8 NeuronCores, batch-parallel.

B=16, T=8192, D=256, U=256. Each core handles 2 batches.

Per-core dataflow (per 512-row tile of values):
  DMA (gpsimd SWDGE) values tile -> SBUF, casting fp32->bf16 in the DMA
  PE  transpose 128x128 bf16 blocks -> PSUM  (values^T)
  DVE copy PSUM -> SBUF                  (moving operand for pass 1)
  PE  pass1: projT[u,t] = W1^T @ valuesT  (all-bf16, W1 stationary)
  ACT tanh(projT + bias[u]) -> bf16      (bias = W1_b + query@W2+W2_b, per-partition)
  PE  V-dot: score[1,t] = V^T @ tanh     (col-tiled: tile q of a group of 4
                                          lands on psum partition 32q)
Per group of 4 tiles:
  DVE copy score psum [128,512] -> SBUF  (rows 0/32/64/96 useful)
  ACT exp(score - bound) + free-dim accumulate -> unnorm weights + row sums
Per batch:
  PE  transpose weight slices -> per-t-block stationary columns (col select)
  PE  pass2: ctx = sum_t w[t] * values[t,:]  (all-bf16, w stationary, N=256)
Host: softmax normalization (divide by row-sum totals), gather across cores.
Build notes: must use bacc.Bacc + nc.compile() (walrus allows only one sync
wait per instruction; bacc legalizes via event semaphores). Engine APs cannot
stride partitions (only DMAs can). float32r normal matmuls are broken on HW.

Softmax uses a constant shift bound=|V_w|_1 instead of the running max
(softmax is shift-invariant; exp(score-bound) <= 1 so no overflow).
"""

import os
import sys

import numpy as np

for _p in ("/opt/trn_rl_repo",):
    if _p not in sys.path and os.path.isdir(_p):
        sys.path.insert(0, _p)

import ml_dtypes  # noqa: E402

B, T, D, U = 16, 8192, 256, 256
N_CORES = 8
BPC = B // N_CORES          # batches per core = 2
JT = T // 512               # 512-row tiles per batch = 16
NJ = BPC * JT               # tiles per core = 32
NG = NJ // 4                # groups of 4 tiles = 8


def build_kernel(shift: float):
    import concourse.bass as bass
    import concourse.mybir as mybir
    import concourse.tile as tile
    from concourse import bacc
    from contextlib import ExitStack

    fp32 = mybir.dt.float32
    fp32r = mybir.dt.float32r
    bf16 = mybir.dt.bfloat16

    nc = bacc.Bacc("TRN2", target_bir_lowering=False)

    vals = nc.dram_tensor("vals", [BPC, T, D], fp32, kind="ExternalInput")
    w1 = nc.dram_tensor("w1", [128, 2, 256], bf16, kind="ExternalInput")
    vw = nc.dram_tensor("vw", [128, 2], bf16, kind="ExternalInput")
    cvec = nc.dram_tensor("cvec", [128, 2 * BPC], fp32, kind="ExternalInput")
    ident = nc.dram_tensor("ident", [128, 128], bf16, kind="ExternalInput")
    ident32 = nc.dram_tensor("ident32", [128, 128], fp32, kind="ExternalInput")
    nshift = nc.dram_tensor("nshift", [128, 1], fp32, kind="ExternalInput")

    aw_out = nc.dram_tensor("aw", [BPC, T], fp32, kind="ExternalOutput")
    rs_out = nc.dram_tensor("rs", [128, NG], fp32, kind="ExternalOutput")
    ctx_out = nc.dram_tensor("ctx", [1, BPC * D], fp32, kind="ExternalOutput")

    with tile.TileContext(nc) as tc:
        # ---- persistent SBUF ----
        v_all = nc.alloc_sbuf_tensor("v_all", [128, NJ, 4, D], bf16).ap()
        w1_sb = nc.alloc_sbuf_tensor("w1_sb", [128, 2, 256], bf16).ap()
        vw_sb = nc.alloc_sbuf_tensor("vw_sb", [128, 2], bf16).ap()
        cvec_sb = nc.alloc_sbuf_tensor("cvec_sb", [128, 2 * BPC], fp32).ap()
        id_sb = nc.alloc_sbuf_tensor("id_sb", [128, 128], bf16).ap()
        id32_sb = nc.alloc_sbuf_tensor("id32_sb", [128, 128], fp32).ap()
        scores_sb = nc.alloc_sbuf_tensor("scores_sb", [128, NG, 512], fp32).ap()
        wexp_sb = nc.alloc_sbuf_tensor("wexp_sb", [128, NG, 512], fp32).ap()
        rs_sb = nc.alloc_sbuf_tensor("rs_sb", [128, NG], fp32).ap()
        nshift_sb = nc.alloc_sbuf_tensor("nshift_sb", [128, 1], fp32).ap()
        wcols_sb = nc.alloc_sbuf_tensor("wcols_sb", [128, BPC, 64], bf16).ap()
        ctx_sb = nc.alloc_sbuf_tensor("ctx_sb", [1, BPC * D], fp32).ap()

        nc.sync.dma_start(w1_sb, w1[:])
        nc.sync.dma_start(vw_sb, vw[:])
        nc.sync.dma_start(cvec_sb, cvec[:])
        nc.sync.dma_start(id_sb, ident[:])
        nc.sync.dma_start(id32_sb, ident32[:])
        nc.sync.dma_start(nshift_sb, nshift[:])

        # values: [b, (j tb p), d] -> v_all[p, b*JT+j, tb, d]
        vsrc = vals.rearrange("b (j tb p) d -> p (b j) tb d", tb=4, p=128)

        pools = ExitStack()
        pT = pools.enter_context(tc.tile_pool(name="pT", bufs=2, space="PSUM"))
        pP = pools.enter_context(tc.tile_pool(name="pP", bufs=3, space="PSUM"))
        pS = pools.enter_context(tc.tile_pool(name="pS", bufs=1, space="PSUM"))
        pM = pools.enter_context(tc.tile_pool(name="pM", bufs=1, space="PSUM"))
        sbT = pools.enter_context(tc.tile_pool(name="sbT", bufs=4))
        sbY = pools.enter_context(tc.tile_pool(name="sbY", bufs=4))

        def do_tile(j, psum_s):
            b = j // JT
            q = j % 4
            # load this tile's values (512 rows), casting fp32->bf16 in the DMA
            nc.gpsimd.dma_start(v_all[:, j], vsrc[:, j])
            # transpose to [d, t] and cast to bf16
            vT = []
            for dh in range(2):
                pt = pT.tile([128, 512], bf16, tag="pt")
                for tb in range(4):
                    nc.tensor.transpose(
                        pt[:, tb * 128:(tb + 1) * 128],
                        v_all[:, j, tb, dh * 128:(dh + 1) * 128],
                        id_sb,
                    )
                vt = sbT.tile([128, 512], bf16, tag="vt")
                nc.vector.tensor_copy(vt, pt)
                vT.append(vt)
            # pass 1: projT[u, t] (+bias) -> tanh -> y (bf16)
            ys = []
            for uh in range(2):
                pp = pP.tile([128, 512], fp32, tag="pp")
                for dh in range(2):
                    nc.tensor.matmul(
                        pp,
                        w1_sb[:, dh, uh * 128:(uh + 1) * 128],
                        vT[dh],
                        start=(dh == 0),
                        stop=(dh == 1),
                    )
                y = sbY.tile([128, 512], bf16, tag="y")
                nc.scalar.activation(
                    y, pp, mybir.ActivationFunctionType.Tanh,
                    bias=cvec_sb[:, b * 2 + uh:b * 2 + uh + 1],
                )
                ys.append(y)
            # V-dot -> score[1, 512] at psum partition 32*q
            for uh in range(2):
                nc.tensor.matmul(
                    psum_s[32 * q:32 * q + 1, :],
                    vw_sb[:, uh:uh + 1],
                    ys[uh],
                    start=(uh == 0),
                    stop=(uh == 1),
                    tile_position=(0, 32 * q),
                )

        def flush_group(g, psum_s):
            # scores of tiles 4g..4g+3 live at psum partitions 0/32/64/96;
            # copy the whole tile (unused rows are garbage, ignored later)
            nc.vector.tensor_copy(scores_sb[:, g, :], psum_s)
            nc.scalar.activation(
                wexp_sb[:, g, :], scores_sb[:, g, :],
                mybir.ActivationFunctionType.Exp,
                bias=nshift_sb,
                accum_out=rs_sb[:, g:g + 1],
            )

        def do_batch_tail(b):
            # attention-weight output rows: t = (4*gg + q)*512 + f
            nc.sync.dma_start(
                aw_out[b].rearrange("(gg q f) -> q gg f", q=4, f=512),
                wexp_sb[0:128:32, 4 * b:4 * b + 4, :],
            )
            # transpose weight slices; select the 4 useful columns per slice
            for gg in range(4):
                g = 4 * b + gg
                for c in range(4):
                    pw = pM.tile([128, 128], fp32, tag="pw")
                    nc.tensor.transpose(
                        pw, wexp_sb[:, g, c * 128:(c + 1) * 128], id32_sb
                    )
                    nc.vector.tensor_copy(
                        wcols_sb.rearrange("p b (s q c) -> p b s q c", q=4, c=4)
                        [:, b, gg, :, c],
                        pw.rearrange("p (a s) -> p a s", s=32)[:, :, 0],
                    )
            # pass 2: ctx[1, 256] = sum_t w[t] * v[t, :]
            pc = pM.tile([1, 256], fp32, tag="pc")
            for m in range(64):
                nc.tensor.matmul(
                    pc,
                    wcols_sb[:, b, m:m + 1],
                    v_all[:, b * JT + m // 4, m % 4, :],
                    start=(m == 0),
                    stop=(m == 63),
                )
            nc.vector.tensor_copy(ctx_sb[:, b * D:(b + 1) * D], pc)

        for b in range(BPC):
            psum_s = None
            for jj in range(JT):
                j = b * JT + jj
                if jj % 4 == 0:
                    psum_s = pS.tile([128, 512], fp32, tag="ps")
                do_tile(j, psum_s)
                if jj % 4 == 3:
                    flush_group(j // 4, psum_s)
            do_batch_tail(b)

        nc.sync.dma_start(rs_out[:], rs_sb)
        nc.sync.dma_start(ctx_out[:], ctx_sb)
        pools.close()

    nc.compile()
    return nc


def kernel(values, query, W1_w, W1_b, W2_w, W2_b, V_w, V_b, _trace=False):
    from concourse.bass_utils import run_bass_kernel_spmd

    values = np.asarray(values, dtype=np.float32)
    query = np.asarray(query, dtype=np.float32)
    W1_w = np.asarray(W1_w, dtype=np.float32)
    W1_b = np.asarray(W1_b, dtype=np.float32)
    W2_w = np.asarray(W2_w, dtype=np.float32)
    W2_b = np.asarray(W2_b, dtype=np.float32)
    V_w = np.asarray(V_w, dtype=np.float32)
    V_b = np.asarray(V_b, dtype=np.float32)

    # host-side tiny precompute
    cvec_full = (query @ W2_w + W2_b + W1_b).astype(np.float32)      # [B, U]
    shift = float(np.abs(V_w).sum())   # score bound (V_b dropped: softmax shift-invariant)

    w1_host = np.ascontiguousarray(
        W1_w.reshape(2, 128, 256).transpose(1, 0, 2)
    ).astype(ml_dtypes.bfloat16)                                      # [128, 2, 256]
    vw_host = np.ascontiguousarray(
        V_w.reshape(2, 128).transpose(1, 0)
    ).astype(ml_dtypes.bfloat16)                                      # [128, 2]
    ident_bf = np.eye(128, dtype=ml_dtypes.bfloat16)
    ident = np.eye(128, dtype=np.float32)

    nc = build_kernel(shift)

    in_maps = []
    for c in range(N_CORES):
        cv = cvec_full[c * BPC:(c + 1) * BPC]                         # [2, 256]
        cvec_host = np.ascontiguousarray(
            cv.reshape(BPC, 2, 128).transpose(2, 0, 1).reshape(128, BPC * 2)
        ).astype(np.float32)                                          # [128, (b,uh)]
        in_maps.append({
            "vals": np.ascontiguousarray(values[c * BPC:(c + 1) * BPC]),
            "w1": w1_host,
            "vw": vw_host,
            "cvec": cvec_host,
            "ident": ident_bf,
            "ident32": ident,
            "nshift": np.full((128, 1), -shift, dtype=np.float32),
        })

    if _trace:
        from concourse.timeline_sim import TimelineSim
        ts = TimelineSim(nc, trace=False)
        ns = ts.simulate()
        print(f"HW exec time: {ns:.0f} ns  (TimelineSim cost model, per core)")
        try:
            import json
            occ = getattr(ts, "device_busy_ns", None) or getattr(ts, "busy_ns", None)
            if occ:
                print("engine busy ns:", {str(k): int(v) for k, v in occ.items()})
        except Exception:
            pass
    res = run_bass_kernel_spmd(nc, in_maps, core_ids=list(range(N_CORES)))

    aw = np.concatenate([r["aw"] for r in res.results], axis=0)       # [B, T]
    rs = np.stack([r["rs"] for r in res.results], axis=0)             # [n_cores, 128, NG]
    ctx = np.concatenate([r["ctx"].reshape(BPC, D) for r in res.results], axis=0)

    # row sum of tile j (global within core) is rs[32*(j%4), j//4]
    q_idx = 32 * (np.arange(JT) % 4)                                  # [16]
    totals = np.zeros(B, dtype=np.float64)
    for c in range(N_CORES):
        for lb in range(BPC):
            g_idx = (lb * JT + np.arange(JT)) // 4
            totals[c * BPC + lb] = rs[c][q_idx, g_idx].astype(np.float64).sum()
    totals = totals.astype(np.float32)

    attention_weights = (aw / totals[:, None])[:, :, None].astype(np.float32)
    context = (ctx / totals[:, None]).astype(np.float32)
    return context, attention_weights


# revision 34
# speedup vs baseline: 1.0077x; 1.0077x over previous
"""Bahdanau attention kernel for Trainium2, 8 NeuronCores, batch-parallel.

B=16, T=8192, D=256, U=256. Each core handles 2 batches.

Per-core dataflow (per 512-row tile of values):
  DMA (gpsimd SWDGE) values tile -> SBUF, casting fp32->bf16 in the DMA
  PE  transpose 128x128 bf16 blocks -> PSUM  (values^T)
  DVE copy PSUM -> SBUF                  (moving operand for pass 1)
  PE  pass1: projT[u,t] = W1^T @ valuesT  (all-bf16, W1 stationary)
  ACT tanh(projT + bias[u]) -> bf16      (bias = W1_b + query@W2+W2_b, per-partition)
  PE  V-dot: score[1,t] = V^T @ tanh     (col-tiled: tile q of a group of 4
                                          lands on psum partition 32q)
Per group of 4 tiles:
  DVE copy score psum [128,512] -> SBUF  (rows 0/32/64/96 useful)
  ACT exp(score - bound) + free-dim accumulate -> unnorm weights + row sums
Per batch:
  PE  transpose weight slices -> per-t-block stationary columns (col select)
  PE  pass2: ctx = sum_t w[t] * values[t,:]  (all-bf16, w stationary, N=256)
Host: softmax normalization (divide by row-sum totals), gather across cores.
Build notes: must use bacc.Bacc + nc.compile() (walrus allows only one sync
wait per instruction; bacc legalizes via event semaphores). Engine APs cannot
stride partitions (only DMAs can). float32r normal matmuls are broken on HW.

Softmax uses a constant shift bound=|V_w|_1 instead of the running max
(softmax is shift-invariant; exp(score-bound) <= 1 so no overflow).
"""

import os
import sys

import numpy as np

for _p in ("/opt/trn_rl_repo",):
    if _p not in sys.path and os.path.isdir(_p):
        sys.path.insert(0, _p)

import ml_dtypes  # noqa: E402

B, T, D, U = 16, 8192, 256, 256
N_CORES = 8
BPC = B // N_CORES          # batches per core = 2
JT = T // 512               # 512-row tiles per batch = 16
NJ = BPC * JT               # tiles per core = 32
NG = NJ // 4                # groups of 4 tiles = 8


def build_kernel(shift: float):
    import concourse.bass as bass
    import concourse.mybir as mybir
    import concourse.tile as tile
    from concourse import bacc
    from contextlib import ExitStack

    fp32 = mybir.dt.float32
    fp32r = mybir.dt.float32r
    bf16 = mybir.dt.bfloat16

    nc = bacc.Bacc("TRN2", target_bir_lowering=False)

    vals = nc.dram_tensor("vals", [BPC, T, D], fp32, kind="ExternalInput")
    w1 = nc.dram_tensor("w1", [128, 2, 256], bf16, kind="ExternalInput")
    vw = nc.dram_tensor("vw", [128, 2], bf16, kind="ExternalInput")
    cvec = nc.dram_tensor("cvec", [128, 2 * BPC], fp32, kind="ExternalInput")
    ident = nc.dram_tensor("ident", [128, 128], bf16, kind="ExternalInput")
    ident32 = nc.dram_tensor("ident32", [128, 128], fp32, kind="ExternalInput")
    nshift = nc.dram_tensor("nshift", [128, 1], fp32, kind="ExternalInput")

    aw_out = nc.dram_tensor("aw", [BPC, T], fp32, kind="ExternalOutput")
    rs_out = nc.dram_tensor("rs", [128, NG], fp32, kind="ExternalOutput")
    ctx_out = nc.dram_tensor("ctx", [1, BPC * D], fp32, kind="ExternalOutput")

    with tile.TileContext(nc) as tc:
        # ---- persistent SBUF ----
        v_all = nc.alloc_sbuf_tensor("v_all", [128, NJ, 4, D], bf16).ap()
        w1_sb = nc.alloc_sbuf_tensor("w1_sb", [128, 2, 256], bf16).ap()
        vw_sb = nc.alloc_sbuf_tensor("vw_sb", [128, 2], bf16).ap()
        cvec_sb = nc.alloc_sbuf_tensor("cvec_sb", [128, 2 * BPC], fp32).ap()
        id_sb = nc.alloc_sbuf_tensor("id_sb", [128, 128], bf16).ap()
        id32_sb = nc.alloc_sbuf_tensor("id32_sb", [128, 128], fp32).ap()
        scores_sb = nc.alloc_sbuf_tensor("scores_sb", [128, NG, 512], fp32).ap()
        wexp_sb = nc.alloc_sbuf_tensor("wexp_sb", [128, NG, 512], fp32).ap()
        rs_sb = nc.alloc_sbuf_tensor("rs_sb", [128, NG], fp32).ap()
        nshift_sb = nc.alloc_sbuf_tensor("nshift_sb", [128, 1], fp32).ap()
        wcols_sb = nc.alloc_sbuf_tensor("wcols_sb", [128, BPC, 64], bf16).ap()
        ctx_sb = nc.alloc_sbuf_tensor("ctx_sb", [1, BPC * D], fp32).ap()

        nc.sync.dma_start(w1_sb, w1[:])
        nc.sync.dma_start(vw_sb, vw[:])
        nc.sync.dma_start(cvec_sb, cvec[:])
        nc.sync.dma_start(id_sb, ident[:])
        nc.sync.dma_start(id32_sb, ident32[:])
        nc.sync.dma_start(nshift_sb, nshift[:])

        # values: [b, (j tb p), d] -> v_all[p, b*JT+j, tb, d]
        vsrc = vals.rearrange("b (j tb p) d -> p (b j) tb d", tb=4, p=128)

        pools = ExitStack()
        pT = pools.enter_context(tc.tile_pool(name="pT", bufs=2, space="PSUM"))
        pP = pools.enter_context(tc.tile_pool(name="pP", bufs=2, space="PSUM"))
        pS = pools.enter_context(tc.tile_pool(name="pS", bufs=2, space="PSUM"))
        pM = pools.enter_context(tc.tile_pool(name="pM", bufs=1, space="PSUM"))
        sbT = pools.enter_context(tc.tile_pool(name="sbT", bufs=4))
        sbY = pools.enter_context(tc.tile_pool(name="sbY", bufs=4))

        def do_load_t(j):
            # load this tile's values (512 rows), casting fp32->bf16 in the
            # DMA, then transpose to [d, t]
            nc.gpsimd.dma_start(v_all[:, j], vsrc[:, j])
            vT = []
            for dh in range(2):
                pt = pT.tile([128, 512], bf16, tag="pt")
                for tb in range(4):
                    nc.tensor.transpose(
                        pt[:, tb * 128:(tb + 1) * 128],
                        v_all[:, j, tb, dh * 128:(dh + 1) * 128],
                        id_sb,
                    )
                vt = sbT.tile([128, 512], bf16, tag="vt")
                nc.vector.tensor_copy(vt, pt)
                vT.append(vt)
            return vT

        def do_tile(j, vT, psum_s):
            b = j // JT
            q = j % 4
            # pass 1: projT[u, t] (+bias) -> tanh -> y (bf16)
            ys = []
            for uh in range(2):
                pp = pP.tile([128, 512], fp32, tag="pp")
                for dh in range(2):
                    nc.tensor.matmul(
                        pp,
                        w1_sb[:, dh, uh * 128:(uh + 1) * 128],
                        vT[dh],
                        start=(dh == 0),
                        stop=(dh == 1),
                    )
                y = sbY.tile([128, 512], bf16, tag="y")
                nc.scalar.activation(
                    y, pp, mybir.ActivationFunctionType.Tanh,
                    bias=cvec_sb[:, b * 2 + uh:b * 2 + uh + 1],
                )
                ys.append(y)
            # V-dot -> score[1, 512] at psum partition 32*q
            for uh in range(2):
                nc.tensor.matmul(
                    psum_s[32 * q:32 * q + 1, :],
                    vw_sb[:, uh:uh + 1],
                    ys[uh],
                    start=(uh == 0),
                    stop=(uh == 1),
                    tile_position=(0, 32 * q),
                )

        def flush_group(g, psum_s):
            # scores of tiles 4g..4g+3 live at psum partitions 0/32/64/96;
            # copy the whole tile (unused rows are garbage, ignored later)
            nc.vector.tensor_copy(scores_sb[:, g, :], psum_s)
            nc.scalar.activation(
                wexp_sb[:, g, :], scores_sb[:, g, :],
                mybir.ActivationFunctionType.Exp,
                bias=nshift_sb,
                accum_out=rs_sb[:, g:g + 1],
            )

        def do_batch_tail(b):
            # attention-weight output rows: t = (4*gg + q)*512 + f
            nc.sync.dma_start(
                aw_out[b].rearrange("(gg q f) -> q gg f", q=4, f=512),
                wexp_sb[0:128:32, 4 * b:4 * b + 4, :],
            )
            # transpose weight slices; select the 4 useful columns per slice
            for gg in range(4):
                g = 4 * b + gg
                for c in range(4):
                    pw = pM.tile([128, 128], fp32, tag="pw")
                    nc.tensor.transpose(
                        pw, wexp_sb[:, g, c * 128:(c + 1) * 128], id32_sb
                    )
                    nc.vector.tensor_copy(
                        wcols_sb.rearrange("p b (s q c) -> p b s q c", q=4, c=4)
                        [:, b, gg, :, c],
                        pw.rearrange("p (a s) -> p a s", s=32)[:, :, 0],
                    )
            # pass 2: ctx[1, 256] = sum_t w[t] * v[t, :]
            pc = pM.tile([1, 256], fp32, tag="pc")
            for m in range(64):
                nc.tensor.matmul(
                    pc,
                    wcols_sb[:, b, m:m + 1],
                    v_all[:, b * JT + m // 4, m % 4, :],
                    start=(m == 0),
                    stop=(m == 63),
                )
            nc.vector.tensor_copy(ctx_sb[:, b * D:(b + 1) * D], pc)

        # software-pipelined: emit tile j+1's transposes before tile j's
        # matmuls so the in-order PE fills cross-engine latency gaps
        psum_s = None
        vT_next = do_load_t(0)
        for j in range(NJ):
            vT = vT_next
            if j + 1 < NJ:
                vT_next = do_load_t(j + 1)
            if j % 4 == 0:
                psum_s = pS.tile([128, 512], fp32, tag="ps")
            do_tile(j, vT, psum_s)
            if j % 4 == 3:
                flush_group(j // 4, psum_s)
            if j % JT == JT - 1:
                do_batch_tail(j // JT)

        nc.sync.dma_start(rs_out[:], rs_sb)
        nc.sync.dma_start(ctx_out[:], ctx_sb)
        pools.close()

    nc.compile()
    return nc


def kernel(values, query, W1_w, W1_b, W2_w, W2_b, V_w, V_b, _trace=False):
    from concourse.bass_utils import run_bass_kernel_spmd

    values = np.asarray(values, dtype=np.float32)
    query = np.asarray(query, dtype=np.float32)
    W1_w = np.asarray(W1_w, dtype=np.float32)
    W1_b = np.asarray(W1_b, dtype=np.float32)
    W2_w = np.asarray(W2_w, dtype=np.float32)
    W2_b = np.asarray(W2_b, dtype=np.float32)
    V_w = np.asarray(V_w, dtype=np.float32)
    V_b = np.asarray(V_b, dtype=np.float32)

    # host-side tiny precompute
    cvec_full = (query @ W2_w + W2_b + W1_b).astype(np.float32)      # [B, U]
    shift = float(np.abs(V_w).sum())   # score bound (V_b dropped: softmax shift-invariant)

    w1_host = np.ascontiguousarray(
        W1_w.reshape(2, 128, 256).transpose(1, 0, 2)
    ).astype(ml_dtypes.bfloat16)                                      # [128, 2, 256]
    vw_host = np.ascontiguousarray(
        V_w.reshape(2, 128).transpose(1, 0)
    ).astype(ml_dtypes.bfloat16)                                      # [128, 2]
    ident_bf = np.eye(128, dtype=ml_dtypes.bfloat16)
    ident = np.eye(128, dtype=np.float32)

    nc = build_kernel(shift)

    in_maps = []
    for c in range(N_CORES):
        cv = cvec_full[c * BPC:(c + 1) * BPC]                         # [2, 256]
        cvec_host = np.ascontiguousarray(
            cv.reshape(BPC, 2, 128).transpose(2, 0, 1).reshape(128, BPC * 2)
        ).astype(np.float32)                                          # [128, (b,uh)]
        in_maps.append({
            "vals": np.ascontiguousarray(values[c * BPC:(c + 1) * BPC]),
            "w1": w1_host,
            "vw": vw_host,
            "cvec": cvec_host,
            "ident": ident_bf,
            "ident32": ident,
            "nshift": np.full((128, 1), -shift, dtype=np.float32),
        })

    if _trace:
        from concourse.timeline_sim import TimelineSim
        ts = TimelineSim(nc, trace=False)
        ns = ts.simulate()
        print(f"HW exec time: {ns:.0f} ns  (TimelineSim cost model, per core)")
        try:
            import json
            occ = getattr(ts, "device_busy_ns", None) or getattr(ts, "busy_ns", None)
            if occ:
                print("engine busy ns:", {str(k): int(v) for k, v in occ.items()})
        except Exception:
            pass
    res = run_bass_kernel_spmd(nc, in_maps, core_ids=list(range(N_CORES)))

    aw = np.concatenate([r["aw"] for r in res.results], axis=0)       # [B, T]
    rs = np.stack([r["rs"] for r in res.results], axis=0)             # [n_cores, 128, NG]
    ctx = np.concatenate([r["ctx"].reshape(BPC, D) for r in res.results], axis=0)

    # row sum of tile j (global within core) is rs[32*(j%4), j//4]
    q_idx = 32 * (np.arange(JT) % 4)                                  # [16]
    totals = np.zeros(B, dtype=np.float64)
    for c in range(N_CORES):
        for lb in range(BPC):
            g_idx = (lb * JT + np.arange(JT)) // 4
            totals[c * BPC + lb] = rs[c][q_idx, g_idx].astype(np.float64).sum()
    totals = totals.astype(np.float32)

    attention_weights = (aw / totals[:, None])[:, :, None].astype(np.float32)
    context = (ctx / totals[:, None]).astype(np.float32)
    return context, attention_weights


# revision 35
# speedup vs baseline: 1.0143x; 1.0066x over previous
"""Bahdanau attention kernel for Trainium2, 8 NeuronCores, batch-parallel.

B=16, T=8192, D=256, U=256. Each core handles 2 batches.

Per-core dataflow (per 512-row tile of values):
  DMA (gpsimd SWDGE) values tile -> SBUF, casting fp32->bf16 in the DMA
  PE  transpose 128x128 bf16 blocks -> PSUM  (values^T)
  DVE copy PSUM -> SBUF                  (moving operand for pass 1)
  PE  pass1: projT[u,t] = W1^T @ valuesT  (all-bf16, W1 stationary)
  ACT tanh(projT + bias[u]) -> bf16      (bias = W1_b + query@W2+W2_b, per-partition)
  PE  V-dot: score[1,t] = V^T @ tanh     (col-tiled: tile q of a group of 4
                                          lands on psum partition 32q)
Per group of 4 tiles:
  DVE copy score psum [128,512] -> SBUF  (rows 0/32/64/96 useful)
  ACT exp(score - bound) + free-dim accumulate -> unnorm weights + row sums
Per batch:
  PE  transpose weight slices -> per-t-block stationary columns (col select)
  PE  pass2: ctx = sum_t w[t] * values[t,:]  (all-bf16, w stationary, N=256)
Host: softmax normalization (divide by row-sum totals), gather across cores.
Build notes: must use bacc.Bacc + nc.compile() (walrus allows only one sync
wait per instruction; bacc legalizes via event semaphores). Engine APs cannot
stride partitions (only DMAs can). float32r normal matmuls are broken on HW.

Softmax uses a constant shift bound=|V_w|_1 instead of the running max
(softmax is shift-invariant; exp(score-bound) <= 1 so no overflow).
"""

import os
import sys

import numpy as np

for _p in ("/opt/trn_rl_repo",):
    if _p not in sys.path and os.path.isdir(_p):
        sys.path.insert(0, _p)

import ml_dtypes  # noqa: E402

B, T, D, U = 16, 8192, 256, 256
N_CORES = 8
BPC = B // N_CORES          # batches per core = 2
JT = T // 512               # 512-row tiles per batch = 16
NJ = BPC * JT               # tiles per core = 32
NG = NJ // 4                # groups of 4 tiles = 8


def build_kernel(shift: float):
    import concourse.bass as bass
    import concourse.mybir as mybir
    import concourse.tile as tile
    from concourse import bacc
    from contextlib import ExitStack

    fp32 = mybir.dt.float32
    fp32r = mybir.dt.float32r
    bf16 = mybir.dt.bfloat16

    nc = bacc.Bacc("TRN2", target_bir_lowering=False)

    vals = nc.dram_tensor("vals", [BPC, T, D], fp32, kind="ExternalInput")
    w1 = nc.dram_tensor("w1", [128, 2, 256], bf16, kind="ExternalInput")
    vw = nc.dram_tensor("vw", [128, 2], bf16, kind="ExternalInput")
    cvec = nc.dram_tensor("cvec", [128, 2 * BPC], fp32, kind="ExternalInput")
    ident = nc.dram_tensor("ident", [128, 128], bf16, kind="ExternalInput")
    ident32 = nc.dram_tensor("ident32", [128, 128], fp32, kind="ExternalInput")
    nshift = nc.dram_tensor("nshift", [128, 1], fp32, kind="ExternalInput")

    aw_out = nc.dram_tensor("aw", [BPC, T], fp32, kind="ExternalOutput")
    rs_out = nc.dram_tensor("rs", [128, NG], fp32, kind="ExternalOutput")
    ctx_out = nc.dram_tensor("ctx", [1, BPC * D], fp32, kind="ExternalOutput")

    with tile.TileContext(nc) as tc:
        # ---- persistent SBUF ----
        v_all = nc.alloc_sbuf_tensor("v_all", [128, NJ, 4, D], bf16).ap()
        w1_sb = nc.alloc_sbuf_tensor("w1_sb", [128, 2, 256], bf16).ap()
        vw_sb = nc.alloc_sbuf_tensor("vw_sb", [128, 2], bf16).ap()
        cvec_sb = nc.alloc_sbuf_tensor("cvec_sb", [128, 2 * BPC], fp32).ap()
        id_sb = nc.alloc_sbuf_tensor("id_sb", [128, 128], bf16).ap()
        id32_sb = nc.alloc_sbuf_tensor("id32_sb", [128, 128], fp32).ap()
        scores_sb = nc.alloc_sbuf_tensor("scores_sb", [128, NG, 512], fp32).ap()
        wexp_sb = nc.alloc_sbuf_tensor("wexp_sb", [128, NG, 512], fp32).ap()
        rs_sb = nc.alloc_sbuf_tensor("rs_sb", [128, NG], fp32).ap()
        nshift_sb = nc.alloc_sbuf_tensor("nshift_sb", [128, 1], fp32).ap()
        wcols_sb = nc.alloc_sbuf_tensor("wcols_sb", [128, BPC, 64], bf16).ap()
        ctx_sb = nc.alloc_sbuf_tensor("ctx_sb", [1, BPC * D], fp32).ap()

        nc.sync.dma_start(w1_sb, w1[:])
        nc.sync.dma_start(vw_sb, vw[:])
        nc.sync.dma_start(cvec_sb, cvec[:])
        nc.sync.dma_start(id_sb, ident[:])
        nc.sync.dma_start(id32_sb, ident32[:])
        nc.sync.dma_start(nshift_sb, nshift[:])

        # values: [b, (j tb p), d] -> v_all[p, b*JT+j, tb, d]
        vsrc = vals.rearrange("b (j tb p) d -> p (b j) tb d", tb=4, p=128)

        pools = ExitStack()
        pT = pools.enter_context(tc.tile_pool(name="pT", bufs=2, space="PSUM"))
        pP = pools.enter_context(tc.tile_pool(name="pP", bufs=2, space="PSUM"))
        pS = pools.enter_context(tc.tile_pool(name="pS", bufs=2, space="PSUM"))
        pM = pools.enter_context(tc.tile_pool(name="pM", bufs=1, space="PSUM"))
        sbT = pools.enter_context(tc.tile_pool(name="sbT", bufs=4))
        sbY = pools.enter_context(tc.tile_pool(name="sbY", bufs=4))

        def do_tile(j, psum_s):
            b = j // JT
            q = j % 4
            # load this tile's values (512 rows), casting fp32->bf16 in the DMA
            nc.gpsimd.dma_start(v_all[:, j], vsrc[:, j])
            # transpose to [d, t] and cast to bf16
            vT = []
            for dh in range(2):
                pt = pT.tile([128, 512], bf16, tag="pt")
                for tb in range(4):
                    nc.tensor.transpose(
                        pt[:, tb * 128:(tb + 1) * 128],
                        v_all[:, j, tb, dh * 128:(dh + 1) * 128],
                        id_sb,
                    )
                vt = sbT.tile([128, 512], bf16, tag="vt")
                nc.vector.tensor_copy(vt, pt)
                vT.append(vt)
            # pass 1: projT[u, t] (+bias) -> tanh -> y (bf16)
            ys = []
            for uh in range(2):
                pp = pP.tile([128, 512], fp32, tag="pp")
                for dh in range(2):
                    nc.tensor.matmul(
                        pp,
                        w1_sb[:, dh, uh * 128:(uh + 1) * 128],
                        vT[dh],
                        start=(dh == 0),
                        stop=(dh == 1),
                    )
                y = sbY.tile([128, 512], bf16, tag="y")
                nc.scalar.activation(
                    y, pp, mybir.ActivationFunctionType.Tanh,
                    bias=cvec_sb[:, b * 2 + uh:b * 2 + uh + 1],
                )
                ys.append(y)
            # V-dot -> score[1, 512] at psum partition 32*q
            for uh in range(2):
                nc.tensor.matmul(
                    psum_s[32 * q:32 * q + 1, :],
                    vw_sb[:, uh:uh + 1],
                    ys[uh],
                    start=(uh == 0),
                    stop=(uh == 1),
                    tile_position=(0, 32 * q),
                )

        def flush_group(g, psum_s):
            # scores of tiles 4g..4g+3 live at psum partitions 0/32/64/96;
            # copy the whole tile (unused rows are garbage, ignored later)
            nc.vector.tensor_copy(scores_sb[:, g, :], psum_s)
            nc.scalar.activation(
                wexp_sb[:, g, :], scores_sb[:, g, :],
                mybir.ActivationFunctionType.Exp,
                bias=nshift_sb,
                accum_out=rs_sb[:, g:g + 1],
            )

        def do_batch_tail(b):
            # attention-weight output rows: t = (4*gg + q)*512 + f
            nc.sync.dma_start(
                aw_out[b].rearrange("(gg q f) -> q gg f", q=4, f=512),
                wexp_sb[0:128:32, 4 * b:4 * b + 4, :],
            )
            # transpose weight slices; select the 4 useful columns per slice
            for gg in range(4):
                g = 4 * b + gg
                for c in range(4):
                    pw = pM.tile([128, 128], fp32, tag="pw")
                    nc.tensor.transpose(
                        pw, wexp_sb[:, g, c * 128:(c + 1) * 128], id32_sb
                    )
                    nc.vector.tensor_copy(
                        wcols_sb.rearrange("p b (s q c) -> p b s q c", q=4, c=4)
                        [:, b, gg, :, c],
                        pw.rearrange("p (a s) -> p a s", s=32)[:, :, 0],
                    )
            # pass 2: ctx[1, 256] = sum_t w[t] * v[t, :]
            pc = pM.tile([1, 256], fp32, tag="pc")
            for m in range(64):
                nc.tensor.matmul(
                    pc,
                    wcols_sb[:, b, m:m + 1],
                    v_all[:, b * JT + m // 4, m % 4, :],
                    start=(m == 0),
                    stop=(m == 63),
                )
            nc.vector.tensor_copy(ctx_sb[:, b * D:(b + 1) * D], pc)

        for b in range(BPC):
            psum_s = None
            for jj in range(JT):
                j = b * JT + jj
                if jj % 4 == 0:
                    psum_s = pS.tile([128, 512], fp32, tag="ps")
                do_tile(j, psum_s)
                if jj % 4 == 3:
                    flush_group(j // 4, psum_s)
            do_batch_tail(b)

        nc.sync.dma_start(rs_out[:], rs_sb)
        nc.sync.dma_start(ctx_out[:], ctx_sb)
        pools.close()

    nc.compile()
    return nc


def kernel(values, query, W1_w, W1_b, W2_w, W2_b, V_w, V_b, _trace=False):
    from concourse.bass_utils import run_bass_kernel_spmd

    values = np.asarray(values, dtype=np.float32)
    query = np.asarray(query, dtype=np.float32)
    W1_w = np.asarray(W1_w, dtype=np.float32)
    W1_b = np.asarray(W1_b, dtype=np.float32)
    W2_w = np.asarray(W2_w, dtype=np.float32)
    W2_b = np.asarray(W2_b, dtype=np.float32)
    V_w = np.asarray(V_w, dtype=np.float32)
    V_b = np.asarray(V_b, dtype=np.float32)

    # host-side tiny precompute
    cvec_full = (query @ W2_w + W2_b + W1_b).astype(np.float32)      # [B, U]
    shift = float(np.abs(V_w).sum())   # score bound (V_b dropped: softmax shift-invariant)

    w1_host = np.ascontiguousarray(
        W1_w.reshape(2, 128, 256).transpose(1, 0, 2)
    ).astype(ml_dtypes.bfloat16)                                      # [128, 2, 256]
    vw_host = np.ascontiguousarray(
        V_w.reshape(2, 128).transpose(1, 0)
    ).astype(ml_dtypes.bfloat16)                                      # [128, 2]
    ident_bf = np.eye(128, dtype=ml_dtypes.bfloat16)
    ident = np.eye(128, dtype=np.float32)

    nc = build_kernel(shift)

    in_maps = []
    for c in range(N_CORES):
        cv = cvec_full[c * BPC:(c + 1) * BPC]                         # [2, 256]
        cvec_host = np.ascontiguousarray(
            cv.reshape(BPC, 2, 128).transpose(2, 0, 1).reshape(128, BPC * 2)
        ).astype(np.float32)                                          # [128, (b,uh)]
        in_maps.append({
            "vals": np.ascontiguousarray(values[c * BPC:(c + 1) * BPC]),
            "w1": w1_host,
            "vw": vw_host,
            "cvec": cvec_host,
            "ident": ident_bf,
            "ident32": ident,
            "nshift": np.full((128, 1), -shift, dtype=np.float32),
        })

    if _trace:
        from concourse.timeline_sim import TimelineSim
        ts = TimelineSim(nc, trace=False)
        ns = ts.simulate()
        print(f"HW exec time: {ns:.0f} ns  (TimelineSim cost model, per core)")
        try:
            import json
            occ = getattr(ts, "device_busy_ns", None) or getattr(ts, "busy_ns", None)
            if occ:
                print("engine busy ns:", {str(k): int(v) for k, v in occ.items()})
        except Exception:
            pass
    res = run_bass_kernel_spmd(nc, in_maps, core_ids=list(range(N_CORES)))

    aw = np.concatenate([r["aw"] for r in res.results], axis=0)       # [B, T]
    rs = np.stack([r["rs"] for r in res.results], axis=0)             # [n_cores, 128, NG]
    ctx = np.concatenate([r["ctx"].reshape(BPC, D) for r in res.results], axis=0)

    # row sum of tile j (global within core) is rs[32*(j%4), j//4]
    q_idx = 32 * (np.arange(JT) % 4)                                  # [16]
    totals = np.zeros(B, dtype=np.float64)
    for c in range(N_CORES):
        for lb in range(BPC):
            g_idx = (lb * JT + np.arange(JT)) // 4
            totals[c * BPC + lb] = rs[c][q_idx, g_idx].astype(np.float64).sum()
    totals = totals.astype(np.float32)

    attention_weights = (aw / totals[:, None])[:, :, None].astype(np.float32)
    context = (ctx / totals[:, None]).astype(np.float32)
    return context, attention_weights


# revision 36
# speedup vs baseline: 1.0211x; 1.0067x over previous
"""Bahdanau attention kernel for Trainium2, 8 NeuronCores, batch-parallel.

B=16, T=8192, D=256, U=256. Each core handles 2 batches.

Per-core dataflow (per 512-row tile of values):
  DMA (gpsimd SWDGE) values tile -> SBUF, casting fp32->bf16 in the DMA
  PE  transpose 128x128 bf16 blocks -> PSUM  (values^T)
  DVE copy PSUM -> SBUF                  (moving operand for pass 1)
  PE  pass1: projT[u,t] = W1^T @ valuesT  (all-bf16, W1 stationary)
  ACT tanh(projT + bias[u]) -> bf16      (bias = W1_b + query@W2+W2_b, per-partition)
  PE  V-dot: score[1,t] = V^T @ tanh     (col-tiled: tile q of a group of 4
                                          lands on psum partition 32q)
Per group of 4 tiles:
  DVE copy score psum [128,512] -> SBUF  (rows 0/32/64/96 useful)
  ACT exp(score - bound) + free-dim accumulate -> unnorm weights + row sums
Per batch:
  PE  transpose weight slices -> per-t-block stationary columns (col select)
  PE  pass2: ctx = sum_t w[t] * values[t,:]  (all-bf16, w stationary, N=256)
Host: softmax normalization (divide by row-sum totals), gather across cores.
Build notes: must use bacc.Bacc + nc.compile() (walrus allows only one sync
wait per instruction; bacc legalizes via event semaphores). Engine APs cannot
stride partitions (only DMAs can). float32r normal matmuls are broken on HW.

Softmax uses a constant shift bound=|V_w|_1 instead of the running max
(softmax is shift-invariant; exp(score-bound) <= 1 so no overflow).
"""

import os
import sys

import numpy as np

for _p in ("/opt/trn_rl_repo",):
    if _p not in sys.path and os.path.isdir(_p):
        sys.path.insert(0, _p)

import ml_dtypes  # noqa: E402

B, T, D, U = 16, 8192, 256, 256
N_CORES = 8
BPC = B // N_CORES          # batches per core = 2
JT = T // 512               # 512-row tiles per batch = 16
NJ = BPC * JT               # tiles per core = 32
NG = NJ // 4                # groups of 4 tiles = 8


def build_kernel(shift: float):
    import concourse.bass as bass
    import concourse.mybir as mybir
    import concourse.tile as tile
    from concourse import bacc
    from contextlib import ExitStack

    fp32 = mybir.dt.float32
    fp32r = mybir.dt.float32r
    bf16 = mybir.dt.bfloat16

    nc = bacc.Bacc("TRN2", target_bir_lowering=False)

    vals = nc.dram_tensor("vals", [BPC, T, D], fp32, kind="ExternalInput")
    w1 = nc.dram_tensor("w1", [128, 2, 256], bf16, kind="ExternalInput")
    vw = nc.dram_tensor("vw", [128, 2], bf16, kind="ExternalInput")
    cvec = nc.dram_tensor("cvec", [128, 2 * BPC], fp32, kind="ExternalInput")
    ident = nc.dram_tensor("ident", [128, 128], bf16, kind="ExternalInput")
    ident32 = nc.dram_tensor("ident32", [128, 128], fp32, kind="ExternalInput")
    nshift = nc.dram_tensor("nshift", [128, 1], fp32, kind="ExternalInput")

    aw_out = nc.dram_tensor("aw", [BPC, T], fp32, kind="ExternalOutput")
    rs_out = nc.dram_tensor("rs", [128, NG], fp32, kind="ExternalOutput")
    ctx_out = nc.dram_tensor("ctx", [1, BPC * D], fp32, kind="ExternalOutput")

    with tile.TileContext(nc) as tc:
        # ---- persistent SBUF ----
        v_all = nc.alloc_sbuf_tensor("v_all", [128, NJ, 4, D], bf16).ap()
        w1_sb = nc.alloc_sbuf_tensor("w1_sb", [128, 2, 256], bf16).ap()
        vw_sb = nc.alloc_sbuf_tensor("vw_sb", [128, 2], bf16).ap()
        cvec_sb = nc.alloc_sbuf_tensor("cvec_sb", [128, 2 * BPC], fp32).ap()
        id_sb = nc.alloc_sbuf_tensor("id_sb", [128, 128], bf16).ap()
        id32_sb = nc.alloc_sbuf_tensor("id32_sb", [128, 128], fp32).ap()
        scores_sb = nc.alloc_sbuf_tensor("scores_sb", [128, NG, 512], fp32).ap()
        wexp_sb = nc.alloc_sbuf_tensor("wexp_sb", [128, NG, 512], fp32).ap()
        rs_sb = nc.alloc_sbuf_tensor("rs_sb", [128, NG], fp32).ap()
        nshift_sb = nc.alloc_sbuf_tensor("nshift_sb", [128, 1], fp32).ap()
        wcols_sb = nc.alloc_sbuf_tensor("wcols_sb", [128, BPC, 64], bf16).ap()
        ctx_sb = nc.alloc_sbuf_tensor("ctx_sb", [1, BPC * D], fp32).ap()

        nc.sync.dma_start(w1_sb, w1[:])
        nc.sync.dma_start(vw_sb, vw[:])
        nc.sync.dma_start(cvec_sb, cvec[:])
        nc.sync.dma_start(id_sb, ident[:])
        nc.sync.dma_start(id32_sb, ident32[:])
        nc.sync.dma_start(nshift_sb, nshift[:])

        # values: [b, (j tb p), d] -> v_all[p, b*JT+j, tb, d]
        vsrc = vals.rearrange("b (j tb p) d -> p (b j) tb d", tb=4, p=128)

        pools = ExitStack()
        pT = pools.enter_context(tc.tile_pool(name="pT", bufs=2, space="PSUM"))
        pP = pools.enter_context(tc.tile_pool(name="pP", bufs=2, space="PSUM"))
        pS = pools.enter_context(tc.tile_pool(name="pS", bufs=2, space="PSUM"))
        pM = pools.enter_context(tc.tile_pool(name="pM", bufs=1, space="PSUM"))
        sbT = pools.enter_context(tc.tile_pool(name="sbT", bufs=4))
        sbY = pools.enter_context(tc.tile_pool(name="sbY", bufs=4))

        def do_tile(j, psum_s):
            b = j // JT
            q = j % 4
            # load this tile's values (512 rows), casting fp32->bf16 in the
            # DMA; the first tile loads in halves so transposes start sooner
            if j == 0:
                nc.gpsimd.dma_start(v_all[:, j, 0:2], vsrc[:, j, 0:2])
                nc.gpsimd.dma_start(v_all[:, j, 2:4], vsrc[:, j, 2:4])
            else:
                nc.gpsimd.dma_start(v_all[:, j], vsrc[:, j])
            # transpose to [d, t] and cast to bf16
            vT = []
            for dh in range(2):
                pt = pT.tile([128, 512], bf16, tag="pt")
                for tb in range(4):
                    nc.tensor.transpose(
                        pt[:, tb * 128:(tb + 1) * 128],
                        v_all[:, j, tb, dh * 128:(dh + 1) * 128],
                        id_sb,
                    )
                vt = sbT.tile([128, 512], bf16, tag="vt")
                nc.vector.tensor_copy(vt, pt)
                vT.append(vt)
            # pass 1: projT[u, t] (+bias) -> tanh -> y (bf16)
            ys = []
            for uh in range(2):
                pp = pP.tile([128, 512], fp32, tag="pp")
                for dh in range(2):
                    nc.tensor.matmul(
                        pp,
                        w1_sb[:, dh, uh * 128:(uh + 1) * 128],
                        vT[dh],
                        start=(dh == 0),
                        stop=(dh == 1),
                    )
                y = sbY.tile([128, 512], bf16, tag="y")
                nc.scalar.activation(
                    y, pp, mybir.ActivationFunctionType.Tanh,
                    bias=cvec_sb[:, b * 2 + uh:b * 2 + uh + 1],
                )
                ys.append(y)
            # V-dot -> score[1, 512] at psum partition 32*q
            for uh in range(2):
                nc.tensor.matmul(
                    psum_s[32 * q:32 * q + 1, :],
                    vw_sb[:, uh:uh + 1],
                    ys[uh],
                    start=(uh == 0),
                    stop=(uh == 1),
                    tile_position=(0, 32 * q),
                )

        def flush_group(g, psum_s):
            # scores of tiles 4g..4g+3 live at psum partitions 0/32/64/96;
            # copy the whole tile (unused rows are garbage, ignored later)
            nc.vector.tensor_copy(scores_sb[:, g, :], psum_s)
            nc.scalar.activation(
                wexp_sb[:, g, :], scores_sb[:, g, :],
                mybir.ActivationFunctionType.Exp,
                bias=nshift_sb,
                accum_out=rs_sb[:, g:g + 1],
            )

        def do_batch_tail(b):
            # attention-weight output rows: t = (4*gg + q)*512 + f
            nc.sync.dma_start(
                aw_out[b].rearrange("(gg q f) -> q gg f", q=4, f=512),
                wexp_sb[0:128:32, 4 * b:4 * b + 4, :],
            )
            # transpose weight slices; select the 4 useful columns per slice
            for gg in range(4):
                g = 4 * b + gg
                for c in range(4):
                    pw = pM.tile([128, 128], fp32, tag="pw")
                    nc.tensor.transpose(
                        pw, wexp_sb[:, g, c * 128:(c + 1) * 128], id32_sb
                    )
                    nc.vector.tensor_copy(
                        wcols_sb.rearrange("p b (s q c) -> p b s q c", q=4, c=4)
                        [:, b, gg, :, c],
                        pw.rearrange("p (a s) -> p a s", s=32)[:, :, 0],
                    )
            # pass 2: ctx[1, 256] = sum_t w[t] * v[t, :]
            pc = pM.tile([1, 256], fp32, tag="pc")
            for m in range(64):
                nc.tensor.matmul(
                    pc,
                    wcols_sb[:, b, m:m + 1],
                    v_all[:, b * JT + m // 4, m % 4, :],
                    start=(m == 0),
                    stop=(m == 63),
                )
            nc.vector.tensor_copy(ctx_sb[:, b * D:(b + 1) * D], pc)
            nc.sync.dma_start(rs_out[:, 4 * b:4 * b + 4], rs_sb[:, 4 * b:4 * b + 4])
            nc.sync.dma_start(ctx_out[:, b * D:(b + 1) * D],
                              ctx_sb[:, b * D:(b + 1) * D])

        for b in range(BPC):
            psum_s = None
            for jj in range(JT):
                j = b * JT + jj
                if jj % 4 == 0:
                    psum_s = pS.tile([128, 512], fp32, tag="ps")
                do_tile(j, psum_s)
                if jj % 4 == 3:
                    flush_group(j // 4, psum_s)
            do_batch_tail(b)

        pools.close()

    nc.compile()
    return nc


def kernel(values, query, W1_w, W1_b, W2_w, W2_b, V_w, V_b, _trace=False):
    from concourse.bass_utils import run_bass_kernel_spmd

    values = np.asarray(values, dtype=np.float32)
    query = np.asarray(query, dtype=np.float32)
    W1_w = np.asarray(W1_w, dtype=np.float32)
    W1_b = np.asarray(W1_b, dtype=np.float32)
    W2_w = np.asarray(W2_w, dtype=np.float32)
    W2_b = np.asarray(W2_b, dtype=np.float32)
    V_w = np.asarray(V_w, dtype=np.float32)
    V_b = np.asarray(V_b, dtype=np.float32)

    # host-side tiny precompute
    cvec_full = (query @ W2_w + W2_b + W1_b).astype(np.float32)      # [B, U]
    shift = float(np.abs(V_w).sum())   # score bound (V_b dropped: softmax shift-invariant)

    w1_host = np.ascontiguousarray(
        W1_w.reshape(2, 128, 256).transpose(1, 0, 2)
    ).astype(ml_dtypes.bfloat16)                                      # [128, 2, 256]
    vw_host = np.ascontiguousarray(
        V_w.reshape(2, 128).transpose(1, 0)
    ).astype(ml_dtypes.bfloat16)                                      # [128, 2]
    ident_bf = np.eye(128, dtype=ml_dtypes.bfloat16)
    ident = np.eye(128, dtype=np.float32)

    nc = build_kernel(shift)

    in_maps = []
    for c in range(N_CORES):
        cv = cvec_full[c * BPC:(c + 1) * BPC]                         # [2, 256]
        cvec_host = np.ascontiguousarray(
            cv.reshape(BPC, 2, 128).transpose(2, 0, 1).reshape(128, BPC * 2)
        ).astype(np.float32)                                          # [128, (b,uh)]
        in_maps.append({
            "vals": np.ascontiguousarray(values[c * BPC:(c + 1) * BPC]),
            "w1": w1_host,
            "vw": vw_host,
            "cvec": cvec_host,
            "ident": ident_bf,
            "ident32": ident,
            "nshift": np.full((128, 1), -shift, dtype=np.float32),
        })

    if _trace:
        from concourse.timeline_sim import TimelineSim
        ts = TimelineSim(nc, trace=False)
        ns = ts.simulate()
        print(f"HW exec time: {ns:.0f} ns  (TimelineSim cost model, per core)")
        try:
            import json
            occ = getattr(ts, "device_busy_ns", None) or getattr(ts, "busy_ns", None)
            if occ:
                print("engine busy ns:", {str(k): int(v) for k, v in occ.items()})
        except Exception:
            pass
    res = run_bass_kernel_spmd(nc, in_maps, core_ids=list(range(N_CORES)))

    aw = np.concatenate([r["aw"] for r in res.results], axis=0)       # [B, T]
    rs = np.stack([r["rs"] for r in res.results], axis=0)             # [n_cores, 128, NG]
    ctx = np.concatenate([r["ctx"].reshape(BPC, D) for r in res.results], axis=0)

    # row sum of tile j (global within core) is rs[32*(j%4), j//4]
    q_idx = 32 * (np.arange(JT) % 4)                                  # [16]
    totals = np.zeros(B, dtype=np.float64)
    for c in range(N_CORES):
        for lb in range(BPC):
            g_idx = (lb * JT + np.arange(JT)) // 4
            totals[c * BPC + lb] = rs[c][q_idx, g_idx].astype(np.float64).sum()
    totals = totals.astype(np.float32)

    attention_weights = (aw / totals[:, None])[:, :, None].astype(np.float32)
    context = (ctx / totals[:, None]).astype(np.float32)
    return context, attention_weights
